# revision 1
# baseline (speedup 1.0000x reference)
"""Trainium2 Bass kernel for nn_AttentionBlock (B=8, L=2048, E=1024, ND=512).

Sharding: data-parallel over batch; 1 batch element per NeuronCore, weights
replicated, no collectives. All heavy matmuls run as float32r (full-rate PE
with ~2e-4 matmul relative error).
"""

import math
import sys

if "/opt/trn_rl_repo" not in sys.path:
    sys.path.insert(0, "/opt/trn_rl_repo")

import numpy as np

import concourse.bass as bass
import concourse.tile as tile
from concourse import bacc, mybir
from concourse.bass_utils import run_bass_kernel_spmd
from concourse.masks import make_identity

F32 = mybir.dt.float32
F32R = mybir.dt.float32r
AF = mybir.ActivationFunctionType
ALU = mybir.AluOpType
AX = mybir.AxisListType

P = 128
E = 1024
ND = 512
F = 2048
LN_EPS = 1e-5
SCALE = math.sqrt(1.0 / E) * 2.0 * math.log(2048)

EC = E // P  # 8 e-chunks
NDC = ND // P  # 4
FC = F // P  # 16


def _bcast(ap, parts=P):
    """Partition-broadcast a 1-D DRAM AP to [parts, n] for DMA."""
    return bass.AP(tensor=ap.tensor, offset=ap.offset, ap=[[0, parts]] + list(ap.ap))


def _layernorm(nc, pool, spool, z, gb, bb, eps_t, out_t):
    """out_t = LN(z) * gb + bb   (z: [P, ND] fp32 SBUF tile, gb/bb: [P, ND] bcast)"""
    nmean = spool.tile([P, 1], F32, tag="nmean")
    nc.vector.reduce_sum(nmean[:], z[:], axis=AX.X)
    nc.vector.tensor_scalar_mul(nmean[:], nmean[:], -1.0 / ND)
    hc = pool.tile([P, ND], F32, tag="ln_hc")
    nc.scalar.activation(hc[:], z[:], AF.Identity, bias=nmean[:])
    sq = pool.tile([P, ND], F32, tag="ln_sq")
    ssq = spool.tile([P, 1], F32, tag="ssq")
    nc.scalar.activation(sq[:], hc[:], AF.Square, accum_out=ssq[:])
    std = spool.tile([P, 1], F32, tag="std")
    nc.scalar.activation(std[:], ssq[:], AF.Sqrt, bias=eps_t[:], scale=1.0 / ND)
    rstd = spool.tile([P, 1], F32, tag="rstd")
    nc.vector.reciprocal(rstd[:], std[:])
    hs = pool.tile([P, ND], F32, tag="ln_hs")
    nc.scalar.activation(hs[:], hc[:], AF.Copy, scale=rstd[:])
    nc.vector.tensor_tensor(out_t[:], hs[:], gb[:], ALU.mult)
    nc.vector.tensor_tensor(out_t[:], out_t[:], bb[:], ALU.add)


def _kernel(tc, L, xT, wqk, wvt, w1T, w2T, b1, b2, g1, be1, g2, be2, out):
    nc = tc.nc
    LT = L // P  # l-tiles of 128
    NB = L // 512  # l-blocks of 512
    SB = L // 512  # score blocks of 512
    W1A = 11  # f-tiles of w1 prefetched during phase 2

    xT_r = xT.rearrange("(c p) l -> p c l", p=P)  # [128, EC, L]
    w1T_r = w1T.rearrange("(c p) f -> p c f", p=P)  # [128, NDC, F]
    w2T_r = w2T.rearrange("(c p) f -> p c f", p=P)  # [128, FC, ND]
    b1_r = b1.rearrange("(t p) -> p t", p=P)  # [128, FC]

    from contextlib import ExitStack

    ctx = ExitStack()
    with ctx:
        ps = ctx.enter_context(tc.tile_pool(name="psum", bufs=8, space="PSUM"))
        dram = ctx.enter_context(tc.tile_pool(name="dram", bufs=1, space="DRAM"))
        const = ctx.enter_context(tc.tile_pool(name="const", bufs=1))
        wff = ctx.enter_context(tc.tile_pool(name="wff", bufs=1))

        qT_d = dram.tile([E, L], F32R)
        h_d = dram.tile([L, ND], F32)
        qT_dr = qT_d.rearrange("(t p) l -> p t l", p=P)  # [128, 8, L]
        h_dr = h_d.rearrange("(b t p) d -> b p t d", t=4, p=P)  # [NB, 128, 4, ND]

        ident = const.tile([P, P], F32)
        make_identity(nc, ident[:])
        ident_r = const.tile([P, P], F32R)
        nc.vector.tensor_copy(ident_r[:], ident[:])
        eps_t = const.tile([P, 1], F32)
        nc.vector.memset(eps_t[:], LN_EPS)
        g1b = const.tile([P, ND], F32)
        nc.sync.dma_start(g1b[:], _bcast(g1))
        be1b = const.tile([P, ND], F32)
        nc.sync.dma_start(be1b[:], _bcast(be1))

        kv_cm = tc.tile_pool(name="kv", bufs=1)
        kv = kv_cm.__enter__()
        kT_sb = kv.tile([P, EC, L], F32R)  # k^T resident
        v_sb = kv.tile([P, LT, ND], F32R)  # v resident

        # ---------------- Phase 1: in-projection ----------------
        with tc.tile_pool(name="pX", bufs=1) as pX:
            xT_sb = pX.tile([P, EC, L], F32R)

            # v = x @ Wv^T  (out [l, nd]; lhsT = xT chunk, rhs = WvT chunk)
            with tc.tile_pool(name="pB", bufs=1) as pB:
                wv = pB.tile([P, EC, ND], F32R)
                # wv in halves + fine first x block: first matmul needs only
                # ~2MB of DMA instead of ~3MB
                nc.sync.dma_start(wv[:, :, :256], wvt[:, :, :256])
                nc.sync.dma_start(xT_sb[:, :, :256], xT_r[:, :, :256])
                nc.sync.dma_start(wv[:, :, 256:], wvt[:, :, 256:])
                nc.sync.dma_start(xT_sb[:, :, 256:512], xT_r[:, :, 256:512])
                for nb in range(1, NB):
                    nc.sync.dma_start(
                        xT_sb[:, :, nb * 512 : (nb + 1) * 512],
                        xT_r[:, :, nb * 512 : (nb + 1) * 512],
                    )
                for ls in range(LT):
                    for vh in range(2):
                        pv = ps.tile([P, 512], F32, tag="ps")
                        for c in range(EC):
                            nc.tensor.matmul(
                                pv[:, :256],
                                xT_sb[:, c, ls * P : (ls + 1) * P],
                                wv[:, c, vh * 256 : (vh + 1) * 256],
                                start=(c == 0),
                                stop=(c == EC - 1),
                            )
                        nc.vector.tensor_copy(
                            v_sb[:, ls, vh * 256 : (vh + 1) * 256], pv[:, :256]
                        )

            # qT / kT = W @ x^T (out [e_out, l]; lhsT = W^T chunk, rhs = xT chunk)
            with (
                tc.tile_pool(name="pA", bufs=2) as pA,
                tc.tile_pool(name="pQ", bufs=3) as pQ,
            ):
                for t in range(16):
                    wt = pA.tile([P, EC, P], F32R, tag="wt")
                    nc.sync.dma_start(wt[:], wqk[t])
                    for nb in range(NB):
                        pq = ps.tile([P, 512], F32, tag="ps")
                        for c in range(EC):
                            nc.tensor.matmul(
                                pq[:],
                                wt[:, c, :],
                                xT_sb[:, c, nb * 512 : (nb + 1) * 512],
                                start=(c == 0),
                                stop=(c == EC - 1),
                            )
                        if t < 8:
                            if nb % 2 == 0:
                                st = pQ.tile([P, 1024], F32R, tag="qst", name="st")
                            nc.vector.tensor_copy(
                                st[:, (nb % 2) * 512 : (nb % 2 + 1) * 512], pq[:]
                            )
                            if nb % 2 == 1 or nb == NB - 1:
                                lo = (nb - nb % 2) * 512
                                nc.sync.dma_start(
                                    qT_dr[:, t, lo : (nb + 1) * 512],
                                    st[:, : (nb + 1) * 512 - lo],
                                )
                        else:
                            nc.vector.tensor_copy(
                                kT_sb[:, t - 8, nb * 512 : (nb + 1) * 512], pq[:]
                            )

        # prefetch most of w1 during attention (phase-1 xT zone is free)
        w1a = wff.tile([P, NDC, W1A * P], F32R)
        nc.sync.dma_start(w1a[:], w1T_r[:, :, : W1A * P])

        # ---------------- Phase 2: attention + LN1 ----------------
        with (
            tc.tile_pool(name="p2", bufs=2) as p2,
            tc.tile_pool(name="p2b", bufs=1) as p2b,
            tc.tile_pool(name="p2s", bufs=4) as p2s,
        ):
            score_ps = {}
            qt_blks = {}

            def emit_scores(lt):
                nb = lt // 4
                if nb not in qt_blks:
                    qtb = p2.tile([P, EC, 512], F32R, tag="qt", name="qtb")
                    if nb == 0:
                        nc.sync.dma_start(qtb[:, :, :256], qT_dr[:, :, :256])
                        nc.sync.dma_start(qtb[:, :, 256:], qT_dr[:, :, 256:512])
                    else:
                        nc.sync.dma_start(
                            qtb[:], qT_dr[:, :, nb * 512 : (nb + 1) * 512]
                        )
                    qt_blks[nb] = qtb
                qt = qt_blks[nb]
                j0 = (lt % 4) * P
                tiles = []
                for sb_ in range(SB):
                    pp = ps.tile([P, 512], F32, tag="ps", name="pp")
                    for c in range(EC):
                        nc.tensor.matmul(
                            pp[:],
                            qt[:, c, j0 : j0 + P],
                            kT_sb[:, c, sb_ * 512 : (sb_ + 1) * 512],
                            start=(c == 0),
                            stop=(c == EC - 1),
                        )
                    tiles.append(pp)
                score_ps[lt] = tiles

            emit_scores(0)
            for lt in range(LT):
                if lt + 1 < LT:
                    emit_scores(lt + 1)  # PE fills the softmax latency
                sc_ps = score_ps.pop(lt)

                # softmax without row-max: scores are far from fp32 exp
                # overflow (|s| < ~40 vs 88), and the reference's
                # max-subtraction is mathematically a no-op.
                p_sb = p2b.tile([P, L], F32R, tag="p")
                sums = []
                for sb_ in range(SB):
                    s_ = p2s.tile([P, 1], F32, tag=f"es{sb_}")
                    nc.scalar.activation(
                        p_sb[:, sb_ * 512 : (sb_ + 1) * 512],
                        sc_ps[sb_][:],
                        AF.Exp,
                        accum_out=s_[:],
                    )
                    sums.append(s_)
                while len(sums) > 1:
                    nxt = []
                    for i in range(0, len(sums) - 1, 2):
                        s_ = p2s.tile([P, 1], F32, tag=f"esr{len(sums)}_{i}")
                        nc.vector.tensor_tensor(
                            s_[:], sums[i][:], sums[i + 1][:], ALU.add
                        )
                        nxt.append(s_)
                    if len(sums) % 2:
                        nxt.append(sums[-1])
                    sums = nxt
                rden = p2s.tile([P, 1], F32, tag="rden")
                nc.vector.reciprocal(rden[:], sums[0][:])

                # transpose P (fp32 PE transpose), store as f32r for attn matmul
                pT = p2b.tile([P, LT, P], F32R, tag="pT")
                for g in range(LT // 4):
                    tp = ps.tile([P, 512], F32R, tag="ps", name="tp")
                    for j in range(4):
                        nc.tensor.transpose(
                            tp[:, j * P : (j + 1) * P],
                            p_sb[:, (g * 4 + j) * P : (g * 4 + j + 1) * P],
                            ident_r[:],
                        )
                    nc.vector.tensor_copy(
                        pT[:, g * 4 : (g + 1) * 4, :],
                        tp[:].rearrange("p (c l) -> p c l", l=P),
                    )

                po = ps.tile([P, 512], F32, tag="ps", name="po")
                for sc in range(LT):
                    nc.tensor.matmul(
                        po[:],
                        pT[:, sc, :],
                        v_sb[:, sc, :],
                        start=(sc == 0),
                        stop=(sc == LT - 1),
                    )
                at = p2.tile([P, ND], F32, tag="at")
                nc.scalar.activation(at[:], po[:], AF.Copy, scale=rden[:])

                h_t = p2.tile([P, ND], F32, tag="h")
                _layernorm(nc, p2, p2s, at, g1b, be1b, eps_t, h_t)
                nc.sync.dma_start(h_d[lt * P : (lt + 1) * P, :], h_t[:])

        kv_cm.__exit__(None, None, None)  # free kT/v before the FF pools open

        # ---------------- Phase 3: FFN + LN2 ----------------
        with (
            tc.tile_pool(name="p3c", bufs=1) as p3c,
            tc.tile_pool(name="p3h", bufs=3) as p3h,
            tc.tile_pool(name="p3", bufs=2) as p3,
            tc.tile_pool(name="p3f", bufs=2) as p3f,
            tc.tile_pool(name="p3s", bufs=4) as p3s,
        ):
            def emit_hb(fb, nm):
                hb = p3h.tile([P, 4, ND], F32, tag="hb", name=nm)
                nc.sync.dma_start(hb[:], h_dr[fb])
                return hb

            first_hb = emit_hb(0, "hb0")
            b1p = p3c.tile([P, FC], F32)
            nc.sync.dma_start(b1p[:], b1_r)
            w1b = p3c.tile([P, NDC, (FC - W1A) * P], F32R)
            nc.sync.dma_start(w1b[:], w1T_r[:, :, W1A * P :])
            w2_sb = p3c.tile([P, FC, ND], F32R)
            # chunked so ff2's fc-ordered accumulation consumes on arrival
            for wq in range(4):
                nc.sync.dma_start(
                    w2_sb[:, wq * 4 : (wq + 1) * 4, :],
                    w2T_r[:, wq * 4 : (wq + 1) * 4, :],
                )
            g2b = p3c.tile([P, ND], F32)
            nc.sync.dma_start(g2b[:], _bcast(g2))
            be2b = p3c.tile([P, ND], F32)
            nc.sync.dma_start(be2b[:], _bcast(be2))
            b2b = p3c.tile([P, ND], F32)
            nc.sync.dma_start(b2b[:], _bcast(b2))

            # hT[nd, l] per l-block via PE transpose of the loaded h block
            def emit_hT(hb):
                hT = p3.tile([P, NDC, 512], F32R, tag="hT", name="hT")
                for t4 in range(4):
                    tp = ps.tile([P, 512], F32, tag="ps", name="tp3")
                    for c in range(NDC):
                        nc.tensor.transpose(
                            tp[:, c * P : (c + 1) * P],
                            hb[:, t4, c * P : (c + 1) * P],
                            ident[:],
                        )
                    nc.vector.tensor_copy(
                        hT[:, :, t4 * P : (t4 + 1) * P],
                        tp[:].rearrange("p (c l) -> p c l", l=P),
                    )
                return hT

            hbs = {0: first_hb}
            hTs = {0: emit_hT(hbs[0])}
            for fb in range(NB):
                hT = hTs.pop(fb)
                hb = hbs.pop(fb)
                if fb + 1 < NB:
                    hbs[fb + 1] = emit_hb(fb + 1, "hbn")
                # ffT = relu(w1 @ hT + b1)   [f, l] layout
                ffT = p3f.tile([P, FC, 512], F32R, tag="ffT")
                for ft in range(FC):
                    pf = ps.tile([P, 512], F32, tag="ps", name="pf")
                    w1s, fo = (w1a, ft) if ft < W1A else (w1b, ft - W1A)
                    for c in range(NDC):
                        nc.tensor.matmul(
                            pf[:],
                            w1s[:, c, fo * P : (fo + 1) * P],
                            hT[:, c, :],
                            start=(c == 0),
                            stop=(c == NDC - 1),
                        )
                    nc.scalar.activation(
                        ffT[:, ft, :], pf[:], AF.Relu, bias=b1p[:, ft : ft + 1]
                    )

                # next block's transposes before ff2 so their DVE copies
                # don't queue behind this block's LN2 chain
                if fb + 1 < NB:
                    hTs[fb + 1] = emit_hT(hbs[fb + 1])

                # ff2 = ffT^T @ w2T ; z = h + ff2 + b2 ; out = LN2(z)
                for t4 in range(4):
                    p2o = ps.tile([P, 512], F32, tag="ps", name="p2o")
                    for fc in range(FC):
                        nc.tensor.matmul(
                            p2o[:],
                            ffT[:, fc, t4 * P : (t4 + 1) * P],
                            w2_sb[:, fc, :],
                            start=(fc == 0),
                            stop=(fc == FC - 1),
                        )
                    z = p3.tile([P, ND], F32, tag="z")
                    nc.vector.tensor_tensor(z[:], p2o[:], hb[:, t4, :], ALU.add)
                    nc.vector.tensor_tensor(z[:], z[:], b2b[:], ALU.add)
                    o_t = p3.tile([P, ND], F32, tag="o")
                    _layernorm(nc, p3, p3s, z, g2b, be2b, eps_t, o_t)
                    row = (fb * 4 + t4) * P
                    nc.sync.dma_start(out[row : row + P, :], o_t[:])



def build_program(L=2048, reps=1):
    nc = bacc.Bacc("TRN2", target_bir_lowering=False, debug=False)
    xT = nc.dram_tensor("xT", [E, L], F32R, kind="ExternalInput").ap()
    wqk = nc.dram_tensor("wqk", [16, P, EC, P], F32R, kind="ExternalInput").ap()
    wvt = nc.dram_tensor("wvt", [P, EC, ND], F32R, kind="ExternalInput").ap()
    w1T = nc.dram_tensor("w1T", [ND, F], F32R, kind="ExternalInput").ap()
    w2T = nc.dram_tensor("w2T", [F, ND], F32R, kind="ExternalInput").ap()
    b1 = nc.dram_tensor("b1", [F], F32, kind="ExternalInput").ap()
    b2 = nc.dram_tensor("b2", [ND], F32, kind="ExternalInput").ap()
    g1 = nc.dram_tensor("g1", [ND], F32, kind="ExternalInput").ap()
    be1 = nc.dram_tensor("be1", [ND], F32, kind="ExternalInput").ap()
    g2 = nc.dram_tensor("g2", [ND], F32, kind="ExternalInput").ap()
    be2 = nc.dram_tensor("be2", [ND], F32, kind="ExternalInput").ap()
    out = nc.dram_tensor("out", [L, ND], F32, kind="ExternalOutput").ap()
    with tile.TileContext(nc) as tc:
        for _ in range(reps):
            _kernel(tc, L, xT, wqk, wvt, w1T, w2T, b1, b2, g1, be1, g2, be2, out)
    nc.compile()
    return nc


def make_in_maps(x, in_proj_w, w1, b1, w2, b2, g1, be1, g2, be2):
    B = x.shape[0]
    xT = np.ascontiguousarray(np.transpose(np.asarray(x, np.float32), (0, 2, 1)))
    wT = np.asarray(in_proj_w, np.float32).T.copy()
    wT[:, :E] *= np.float32(SCALE)
    # qk weights tiled [t, p, c, j] so each wt DMA reads 4KB-contiguous lines
    wqk = np.ascontiguousarray(
        wT[:, : 2 * E].reshape(EC, P, 16, P).transpose(2, 1, 0, 3)
    )
    # v weights tiled [p, c, j]
    wvt = np.ascontiguousarray(wT[:, 2 * E :].reshape(EC, P, ND).transpose(1, 0, 2))
    w1T = np.ascontiguousarray(np.asarray(w1, np.float32).T)
    w2T = np.ascontiguousarray(np.asarray(w2, np.float32).T)
    common = dict(
        wqk=wqk,
        wvt=wvt,
        w1T=w1T,
        w2T=w2T,
        b1=np.asarray(b1, np.float32),
        b2=np.asarray(b2, np.float32),
        g1=np.asarray(g1, np.float32),
        be1=np.asarray(be1, np.float32),
        g2=np.asarray(g2, np.float32),
        be2=np.asarray(be2, np.float32),
    )
    return [dict(xT=xT[b], **common) for b in range(B)]


def kernel(**inputs):
    in_maps = make_in_maps(**inputs)
    nc = build_program()
    res = run_bass_kernel_spmd(nc, in_maps, list(range(len(in_maps))))
    return np.stack([r["out"] for r in res.results], axis=0)


if __name__ == "__main__":
    nc = build_program()
    print("built ok")

